# revision 39
# baseline (speedup 1.0000x reference)
"""CodebookLinear TRN2 kernel, v19: gather-paced W-stationary matmul.

Reference computation (jax):
    W = codebook[indices].reshape(-1)[:4096*4096].reshape(4096, 4096)   # [out, in]
    out = einsum('bsi,oi->bso', x, W) + bias

Distribution: 8 NeuronCores, column-parallel over out_features (each core
owns 512 output features and all 8192 tokens), no collectives.

The kernel is GpSimd-bound: ap_gather reconstructs W^T at ~28ns/index
(hardware floor for the instruction: ~2 indices per Q7 RD_CMD and
ReadOverlap=0 on cayman), ~906us/core total; everything else must hide
under it without ever back-pressuring the gather queue.

  gather:   32 full k-tile ap_gathers (1024 idx each, ~28.4us) run
            back-to-back on the Pool queue, paced by the g2 pool via the
            software-pipelined select emissions.
  select:   parity select g2[p, 2o+(p&1)] -> WT[it] bf16 on DVE
            (tensor_copy + copy_predicated). Each select is emitted ~one
            token-group AFTER its gather completes so it never stalls the
            PSUM drains queued behind it (the in-order DVE queue runs
            ahead of GpSimd; a select parked on an unfinished gather
            would freeze PSUM recycling and cool the PE clock-gate).
  matmul:   out^T[o, t]: lhsT = W^T slice [128k, 128o] (stationary),
            rhs = x^T [128k, 512t] bf16 halves. One PSUM unit per
            (token group, ot): a [128, 1024] tile spanning 2 banks that
            the matmuls fill in single-bank halves and the DVE drains in
            ONE tensor_tensor op -> 4 units in flight across the 8 banks
            and half the drain instructions. k accumulates in PSUM per
            chunk pass ([2]*16); passes add into a bf16 out^T
            accumulator; the final pass adds acc + stores f32.
  x feed:   per (k-tile, token-group) [128, 1024] f32 tiles on the sync
            queue (10-deep pool, 4KB packets), cast to bf16 on Scalar.
            Out DMA on the GpSimd SWDGE queue (idle after the gathers).

Host side only shards/reshapes: x transposed + row-permuted, indices
int16 in the wrapped per-group interleaved layout (pure permutation),
codebook transposed/duplicated into the gather's SBUF layout, bias
sliced per-partition. Kernel returns out^T [512, 8192] per core; host
transposes into [4, 2048, 4096].

Index/partition math (per core, o local in [0, O_LOC)):
  Within k-tile it, SBUF partition p holds contraction row
      i = 128*it + sigma(p),  sigma(p) = 8*(2*(p>>4) + (p&1)) + ((p>>1)&7)
  so  j(i) = 16*it + 2*g + h,  k(i) = (p>>1)&7,  g = p>>4,  h = p&1.
  group g's list for k-tile it:  L[n = 2*o + h] = idx[o, 16*it + 2*g + h]
  wrapped storage:               idxw[16*g + q, it, f] = L[16*f + q]
  gather:  g2[p, n] = data[p, L[g(p)][n]] = cb[idx[o(n), j], k(p)]
  select:  W^T[p, o] = g2[p, 2*o + (p&1)]
"""

import sys

for _p in ("/opt/trn_rl_repo",):
    if _p not in sys.path:
        sys.path.insert(0, _p)

import numpy as np

import concourse.bacc as bacc
import concourse.mybir as mybir
import concourse.tile as tile
from concourse.bass_utils import run_bass_kernel_spmd

# Problem constants
OUT_F = 4096
IN_F = 4096
KCB = 4096          # codebook entries
BS = 8              # block size
JB = IN_F // BS     # 512 blocks per W row
B, S = 4, 2048
T = B * S           # 8192 tokens

# Shard grid: column-parallel over out_features
S_O = 8
O_LOC = OUT_F // S_O   # 512
T_LOC = T              # all tokens on every core

P = 128
NIT = IN_F // P        # 32 k-tiles
NOT = O_LOC // P       # 4 out tiles
FW = 2 * O_LOC // 16   # 64 wrapped index columns per k-tile

TG = 1024              # tokens per x tile / PSUM unit width
NTG = T_LOC // TG      # 8 token groups

# k-chunk passes (sum = NIT); small tail chunks shrink the post-gather tail
CHUNKS = [2] * 16
NCH = len(CHUNKS)
CH0 = [sum(CHUNKS[:i]) for i in range(NCH)]
GLOOK = 4              # gather emission lookahead (g2 pool has 6 bufs)

# partition -> within-tile contraction row
_p_ar = np.arange(P)
SIGMA = (8 * (2 * (_p_ar >> 4) + (_p_ar & 1)) + ((_p_ar >> 1) & 7)).astype(np.int64)

_nc_cache = None
last_result = None     # BassKernelResults of the most recent run (for test.py)


def build_nc():
    nc = bacc.Bacc("TRN2", target_bir_lowering=False, debug=False)
    xT = nc.dram_tensor("xT", [IN_F, T_LOC], mybir.dt.float32, kind="ExternalInput")
    idxw = nc.dram_tensor("idxw", [P, NIT * FW], mybir.dt.int16, kind="ExternalInput")
    dataf = nc.dram_tensor("dataf", [P, KCB], mybir.dt.float32,
                           kind="ExternalInput")
    bias = nc.dram_tensor("bias", [P, NOT], mybir.dt.float32, kind="ExternalInput")
    mask = nc.dram_tensor("mask", [P, 1], mybir.dt.uint8, kind="ExternalInput")
    out = nc.dram_tensor("out", [O_LOC, T_LOC], mybir.dt.float32,
                         kind="ExternalOutput")

    with tile.TileContext(nc) as tc:
        with (
            tc.tile_pool(name="const", bufs=1) as constp,
            tc.tile_pool(name="wt", bufs=1) as wtp,
            tc.tile_pool(name="acc", bufs=1) as accp,
            tc.tile_pool(name="g2p", bufs=6) as g2p,
            tc.tile_pool(name="xfp", bufs=10) as xfp,
            tc.tile_pool(name="xbp", bufs=6) as xbp,
            tc.tile_pool(name="outp", bufs=3) as outp,
            tc.tile_pool(name="psmm", bufs=4, space="PSUM") as psmm,
        ):
            # gather inputs first: the first ap_gather is the critical path
            # cb columns per partition, host-prepared: data[p] = cb[:, k(p)]
            data = constp.tile([P, KCB], mybir.dt.float32)
            nc.sync.dma_start(out=data[:], in_=dataf[:, :])
            # indices in 8 slices so gather 0 only waits for slice 0
            idxt = constp.tile([P, NIT * FW], mybir.dt.int16)
            for q in range(8):
                iq = slice(q * NIT * FW // 8, (q + 1) * NIT * FW // 8)
                nc.sync.dma_start(out=idxt[:, iq], in_=idxw[:, iq])
            bias_t = constp.tile([P, NOT], mybir.dt.float32)
            nc.sync.dma_start(out=bias_t[:], in_=bias[:, :])
            mask_t = constp.tile([P, 1], mybir.dt.uint8)
            nc.sync.dma_start(out=mask_t[:], in_=mask[:, :])

            # W^T resident, bf16, one tile per k-tile: [sigma-row, o]
            WT = [
                wtp.tile([P, O_LOC], mybir.dt.bfloat16, name=f"WT{it}")
                for it in range(NIT)
            ]
            # out^T accumulator, bf16, one tile per (token group, out tile)
            acc = [
                accp.tile([P, TG], mybir.dt.bfloat16, name=f"acc{i}")
                for i in range(NTG * NOT)
            ]

            mask_bc = mask_t[:, 0:1].to_broadcast([P, O_LOC])
            xTr = xT[:, :].rearrange("(it p) t -> p it t", p=P)  # [128, NIT, T]

            g2_tiles = {}

            def gather_full(it):
                """ap_gather for one whole k-tile (Pool queue)."""
                g2 = g2p.tile([P, 2 * O_LOC], mybir.dt.float32, name="g2")
                g2_tiles[it] = g2
                nc.gpsimd.ap_gather(
                    out_ap=g2[:, :],
                    in_ap=data[:, :],
                    idxs_ap=idxt[:, it * FW : (it + 1) * FW],
                    channels=P,
                    num_elems=KCB,
                    d=1,
                    num_idxs=2 * O_LOC,
                )

            def select_full(it):
                """parity select -> WT[it] (DVE)."""
                g2 = g2_tiles.pop(it)
                g2_s = g2[:, :].rearrange("p (o s) -> p o s", s=2)
                dst = WT[it][:, :]
                nc.vector.tensor_copy(out=dst, in_=g2_s[:, :, 0])
                nc.vector.copy_predicated(out=dst, mask=mask_bc, data=g2_s[:, :, 1])

            state = {"g": 0}

            def emit_gather():
                if state["g"] < NIT:
                    gather_full(state["g"])
                    state["g"] += 1

            def emit_select(n):
                select_full(n)
                emit_gather()

            sel_queue = []

            def mm_pass(c):
                """Matmul pass for k-tile chunk c over all token groups.

                Per (tg, quarter): 4 PSUM tiles [128, 256] (one per ot)
                accumulate the chunk's k-tiles; 16 PSUM tiles total give
                unit pipeline depth 4, so a late drain cannot stall the
                next unit's start-matmuls. Drains on DVE; x casts on
                Scalar; next chunk's selects interleave one tg late.
                """
                first, last = (c == 0), (c == NCH - 1)
                ch = CHUNKS[c]
                k0 = CH0[c]
                for tg in range(NTG):
                    xbs = {}
                    # one unit per ot: a [128, 1024] PSUM tile spanning 2
                    # banks (matmuls write 512-wide single-bank halves, the
                    # DVE drains both banks in ONE op). 4 units in flight
                    # across the 8 banks, so a late drain cannot stall the
                    # next unit's start-matmuls.
                    for ot in range(NOT):
                        ps = psmm.tile([P, TG], mybir.dt.float32, name="ps")
                        for itl in range(ch):
                            it = k0 + itl
                            if ot == 0:
                                xf = xfp.tile([P, TG], mybir.dt.float32,
                                              name="xf")
                                nc.sync.dma_start(
                                    out=xf[:, :],
                                    in_=xTr[:, it, tg * TG : (tg + 1) * TG],
                                )
                                xb = xbp.tile([P, TG], mybir.dt.bfloat16,
                                              name="xb")
                                nc.scalar.copy(out=xb[:, :], in_=xf[:, :])
                                xbs[it] = xb
                            for hfi in range(2):
                                hs = slice(hfi * (TG // 2), (hfi + 1) * (TG // 2))
                                nc.tensor.matmul(
                                    out=ps[:, hs],
                                    lhsT=WT[it][:, ot * P : (ot + 1) * P],
                                    rhs=xbs[it][:, hs],
                                    start=(itl == 0),
                                    stop=(itl == ch - 1),
                                )
                        a = acc[tg * NOT + ot]
                        if first:
                            nc.vector.tensor_tensor(
                                out=a[:, :], in0=ps[:],
                                in1=bias_t[:, ot : ot + 1].to_broadcast([P, TG]),
                                op=mybir.AluOpType.add,
                            )
                        elif not last:
                            nc.vector.tensor_tensor(
                                out=a[:, :], in0=ps[:], in1=a[:, :],
                                op=mybir.AluOpType.add,
                            )
                        else:
                            outt = outp.tile([P, TG], mybir.dt.float32,
                                             name="outt")
                            nc.vector.tensor_tensor(
                                out=outt[:, :], in0=ps[:], in1=a[:, :],
                                op=mybir.AluOpType.add,
                            )
                            nc.gpsimd.dma_start(
                                out=out[ot * P : (ot + 1) * P,
                                        tg * TG : (tg + 1) * TG],
                                in_=outt[:],
                            )
                    # next chunk's selects, one tg late so the DVE never
                    # parks on an unfinished gather ahead of queued drains
                    if tg > 0 and sel_queue:
                        emit_select(sel_queue.pop(0))

            for _ in range(GLOOK):
                emit_gather()
            for n in range(CHUNKS[0]):
                emit_select(n)
            for c in range(NCH):
                if c + 1 < NCH:
                    sel_queue.extend(
                        range(CH0[c + 1], CH0[c + 1] + CHUNKS[c + 1])
                    )
                mm_pass(c)
                while sel_queue:
                    emit_select(sel_queue.pop(0))

    nc.compile()
    return nc


def _get_nc():
    global _nc_cache
    if _nc_cache is None:
        _nc_cache = build_nc()
    return _nc_cache


def _wrap_indices(idx_local):
    """[O_LOC, JB] int -> wrapped interleaved int16 [P, NIT*FW]."""
    arr = idx_local.reshape(O_LOC, NIT, 8, 2)        # [o, it, g, h]
    L = arr.transpose(2, 1, 0, 3).reshape(8, NIT, 2 * O_LOC)   # [g, it, n=2o+h]
    Lw = L.reshape(8, NIT, FW, 16)                   # [g, it, f, q]
    idxw = Lw.transpose(0, 3, 1, 2).reshape(P, NIT * FW)
    return np.ascontiguousarray(idxw.astype(np.int16))


def make_in_maps(x, codebook, indices, bias):
    x = np.asarray(x, dtype=np.float32).reshape(T, IN_F)
    xT_full = np.ascontiguousarray(x.T)  # [IN_F, T]
    # permute contraction rows within each 128-tile to match the W^T layout
    xT_perm = np.ascontiguousarray(
        xT_full.reshape(NIT, P, T)[:, SIGMA, :].reshape(IN_F, T)
    )
    idx2d = np.asarray(indices).astype(np.int64).reshape(OUT_F, JB)
    cb = np.asarray(codebook, dtype=np.float32)      # [4096, 8]
    # data[p] = cb[:, k(p)],  k(p) = (p>>1)&7
    kcol = ((np.arange(P) >> 1) & 7).astype(np.int64)
    dataf_host = np.ascontiguousarray(cb.T[kcol])    # [128, 4096]
    b = np.asarray(bias, dtype=np.float32)
    mask_np = (np.arange(P) % 2).astype(np.uint8).reshape(P, 1)

    in_maps = []
    for c in range(8):
        # bias per partition: bias_t[p, ot] = bias[c*O_LOC + ot*128 + p]
        bl = b[c * O_LOC : (c + 1) * O_LOC].reshape(NOT, P).T
        in_maps.append(
            {
                "xT": xT_perm,
                "idxw": _wrap_indices(idx2d[c * O_LOC : (c + 1) * O_LOC]),
                "dataf": dataf_host,
                "bias": np.ascontiguousarray(bl),
                "mask": mask_np,
            }
        )
    return in_maps


def assemble(outs):
    full = np.empty((T, OUT_F), dtype=np.float32)
    for c in range(8):
        full[:, c * O_LOC : (c + 1) * O_LOC] = outs[c]["out"].T
    return full.reshape(B, S, OUT_F)


def kernel(x, codebook, indices, bias):
    global last_result
    nc = _get_nc()
    in_maps = make_in_maps(x, codebook, indices, bias)
    last_result = run_bass_kernel_spmd(nc, in_maps, core_ids=list(range(8)))
    return assemble(last_result.results)
